# revision 29
# baseline (speedup 1.0000x reference)
"""Trainium2 Bass kernel for causal multi-head attention (B=4, S=2048, D=512, H=8).

Returns (out, attn) like the reference. Sharding: 8 cores = 4 batches x 2
head-groups; each core handles 1 batch x 4 heads. The [B,H,Sq,Sk] attn tensor
shards naturally along (B, H); the output projection is computed per-core as a
partial over its 4 heads and summed on the host during the gather.

All matmul operands use float32r (fp32 storage, reduced-precision multiply,
full-rate on the PE at free-dim >= 256); softmax runs in fp32 on the scalar
engine (exp with accumulated row sums). The causal mask is exploited
structurally: all-zero 128x128 blocks of the mask are skipped entirely (the
output buffer is zero-initialized), all-ones blocks skip masking, and mixed
blocks get an additive -1e30 mask before exp.

Self-contained: hardcodes shapes; builds + compiles the Bass program on first
call, runs via bass_utils.run_bass_kernel_spmd on cores 0-7.
"""

import numpy as np

import concourse.bass as bass
import concourse.mybir as mybir
import concourse.tile as tile
from concourse import bacc, bass_utils
from concourse.masks import make_identity

B, S, D, H = 4, 2048, 512, 8
DK = D // H                    # 64 head dim
NCORES = 8
HPC = H // 2                   # 4 heads per core
P = 128                        # partition tile
NQT = S // P                   # 16 row/col tiles
NDC = D // P                   # 4 input-dim chunks
CHUNK_BLKS = 8                 # 8x128 = 1024-wide PSUM chunks (2 banks)
SG = 256                       # projection s-group width
SCALE = 1.0 / 8.0              # 1/sqrt(DK)
NEG = -1.0e30

F32 = mybir.dt.float32
F32R = mybir.dt.float32r
BF = mybir.dt.bfloat16
AF = mybir.ActivationFunctionType

_CACHE: dict = {}


# ---------------------------------------------------------------- mask prep

def _mask_structure(mask2d: np.ndarray):
    """Classify each 128x128 block of the [S,S] mask."""
    blk = mask2d.reshape(NQT, P, NQT, P).transpose(0, 2, 1, 3)
    nz = blk.any(axis=(2, 3))
    allone = blk.all(axis=(2, 3))
    mixed = nz & ~allone
    mix_idx = {}
    mix_add, mixT_add = [], []
    for qb in range(NQT):
        for kb in range(NQT):
            if mixed[qb, kb]:
                mix_idx[(qb, kb)] = len(mix_add)
                add = np.where(blk[qb, kb] == 0, np.float32(NEG), np.float32(0.0))
                mix_add.append(add)
                mixT_add.append(np.ascontiguousarray(add.T))
    n = len(mix_add)
    mix_np = np.stack(mix_add) if n else np.zeros((0, P, P), np.float32)
    mixT_np = np.stack(mixT_add) if n else np.zeros((0, P, P), np.float32)
    return nz, mixed, mix_idx, mix_np, mixT_np


def _runs(idxs):
    """Split a sorted index list into maximal consecutive runs."""
    runs = []
    for i in idxs:
        if runs and i == runs[-1][-1] + 1:
            runs[-1].append(i)
        else:
            runs.append([i])
    return runs


# ---------------------------------------------------------------- program

def _build_program(nz, mixed, mix_idx, n_mix):
    nc = bacc.Bacc("TRN2", target_bir_lowering=False, debug=False,
                   num_devices=NCORES)

    # per-core external IO (f32r tensors carry plain fp32 bytes)
    qT_d = nc.dram_tensor("qT", [NDC, P, S], F32R, kind="ExternalInput")
    kT_d = nc.dram_tensor("kT", [NDC, P, S], F32R, kind="ExternalInput")
    vT_d = nc.dram_tensor("vT", [NDC, P, S], F32R, kind="ExternalInput")
    wq_d = nc.dram_tensor("wq", [NDC, P, HPC * DK], F32R, kind="ExternalInput")
    wk_d = nc.dram_tensor("wk", [NDC, P, HPC * DK], F32R, kind="ExternalInput")
    wv_d = nc.dram_tensor("wv", [NDC, P, HPC * DK], F32R, kind="ExternalInput")
    wo_d = nc.dram_tensor("wo", [HPC, DK, D], F32R, kind="ExternalInput")
    bq_d = nc.dram_tensor("bq2", [P, 2], F32, kind="ExternalInput")
    bk_d = nc.dram_tensor("bk2", [P, 2], F32, kind="ExternalInput")
    if n_mix:
        mm_d = nc.dram_tensor("mmix", [n_mix, P, P], F32, kind="ExternalInput")
        mmT_d = nc.dram_tensor("mmixT", [n_mix, P, P], F32, kind="ExternalInput")
    attn_o = nc.dram_tensor("attn_o", [HPC, S, S], F32, kind="ExternalOutput")
    out_p = nc.dram_tensor("out_p", [S, D], F32, kind="ExternalOutput")

    klist = [[kb for kb in range(NQT) if nz[qb, kb]] for qb in range(NQT)]
    qlist = [[qb for qb in range(NQT) if nz[qb, kb]] for kb in range(NQT)]
    first_kb = {qb: klist[qb][0] for qb in range(NQT) if klist[qb]}
    last_kb = {qb: klist[qb][-1] for qb in range(NQT) if klist[qb]}

    with tile.TileContext(nc) as tc:
        with (
            tc.tile_pool(name="const", bufs=1) as const,
            tc.tile_pool(name="stream", bufs=2) as stream,
            tc.tile_pool(name="proj", bufs=1) as proj,
            tc.tile_pool(name="apool", bufs=3) as apool,
            tc.tile_pool(name="atpool", bufs=4) as atpool,
            tc.tile_pool(name="otpool", bufs=4) as otpool,
            tc.tile_pool(name="rpool", bufs=2) as rpool,
            tc.tile_pool(name="small", bufs=6) as small,
            tc.tile_pool(name="opool", bufs=3) as opool,
            tc.tile_pool(name="ps", bufs=3, space="PSUM") as ps,
            tc.tile_pool(name="po", bufs=1, space="PSUM") as po,
            tc.tile_pool(name="dram", bufs=4, space="DRAM") as dram,
        ):
            # ---- constants
            wq_sb = const.tile([P, NDC, HPC * DK], F32R, tag="wq")
            wk_sb = const.tile([P, NDC, HPC * DK], F32R, tag="wk")
            wv_sb = const.tile([P, NDC, HPC * DK], F32R, tag="wv")
            wo_sb = const.tile([DK, HPC, D], F32R, tag="wo")
            bq_sb = const.tile([P, 2], F32, tag="bq")
            bk_sb = const.tile([P, 2], F32, tag="bk")
            ident = const.tile([P, P], F32, tag="ident")
            nc.sync.dma_start(wq_sb[:], wq_d[:].rearrange("c p m -> p c m"))
            nc.sync.dma_start(wk_sb[:], wk_d[:].rearrange("c p m -> p c m"))
            nc.sync.dma_start(wv_sb[:], wv_d[:].rearrange("c p m -> p c m"))
            nc.sync.dma_start(wo_sb[:], wo_d[:].rearrange("h p d -> p h d"))
            nc.sync.dma_start(bq_sb[:], bq_d[:])
            nc.sync.dma_start(bk_sb[:], bk_d[:])
            make_identity(nc, ident[:])
            if n_mix:
                mm_sb = const.tile([P, n_mix, P], BF, tag="mm")
                mmT_sb = const.tile([P, n_mix, P], BF, tag="mmT")
                nc.gpsimd.dma_start(mm_sb[:], mm_d[:].rearrange("n p k -> p n k"))
                nc.gpsimd.dma_start(mmT_sb[:], mmT_d[:].rearrange("n p k -> p n k"))

            # ---- projections, streaming the transposed inputs per s-group
            QT = proj.tile([P, 2, S], F32R, tag="QT")
            KT = proj.tile([P, 2, S], F32R, tag="KT")
            V = proj.tile([P, NQT, HPC * DK], F32R, tag="V")
            NSG = S // SG

            def emit_proj_v(sg):
                sl = slice(sg * SG, (sg + 1) * SG)
                vts = stream.tile([P, NDC, SG], F32R, tag="vts")
                nc.sync.dma_start(vts[:], vT_d[:, :, sl].rearrange("c p s -> p c s"))
                for t in range(SG // P):
                    st = sg * (SG // P) + t
                    pv = ps.tile([P, 1024], F32, tag="ps")
                    for c in range(NDC):
                        nc.tensor.matmul(
                            pv[:, :HPC * DK], vts[:, c, t * P:(t + 1) * P],
                            wv_sb[:, c, :], start=(c == 0), stop=(c == NDC - 1))
                    nc.vector.tensor_copy(V[:, st, :], pv[:, :HPC * DK])

            def emit_proj(sg):
                sl = slice(sg * SG, (sg + 1) * SG)
                qts = stream.tile([P, NDC, SG], F32R, tag="qts")
                kts = stream.tile([P, NDC, SG], F32R, tag="kts")
                nc.sync.dma_start(qts[:], qT_d[:, :, sl].rearrange("c p s -> p c s"))
                nc.sync.dma_start(kts[:], kT_d[:, :, sl].rearrange("c p s -> p c s"))
                for pair in range(2):
                    psl = slice(pair * P, (pair + 1) * P)
                    pq = ps.tile([P, 1024], F32, tag="ps")
                    pk = ps.tile([P, 1024], F32, tag="ps")
                    for c in range(NDC):
                        nc.tensor.matmul(pq[:, :SG], wq_sb[:, c, psl], qts[:, c, :],
                                         start=(c == 0), stop=(c == NDC - 1))
                    nc.vector.tensor_scalar_add(QT[:, pair, sl], pq[:, :SG],
                                                bq_sb[:, pair:pair + 1])
                    for c in range(NDC):
                        nc.tensor.matmul(pk[:, :SG], wk_sb[:, c, psl], kts[:, c, :],
                                         start=(c == 0), stop=(c == NDC - 1))
                    nc.vector.tensor_scalar_add(KT[:, pair, sl], pk[:, :SG],
                                                bk_sb[:, pair:pair + 1])

            # ---- attention emission units. Each unit carries the highest
            # projection s-group it reads so head 0 can interleave with the
            # projection/input-load stream; within each (head, q-half) the
            # loop1 (attn output) and loop2 (A^T + PV) streams are woven so
            # every engine's static order alternates between them.
            OT = [None] * HPC
            HQ = NQT // 2
            units = []                     # (need_sg, emit_fn) in order

            def emit_outproj(half):
                # out[q,:] = sum_h OT_h[:,q].T @ wo_h for this q-half
                for qb in range(half * HQ, (half + 1) * HQ):
                    pso = ps.tile([P, 1024], F32, name="pso", tag="ps")
                    for hh in range(HPC):
                        nc.tensor.matmul(
                            pso[:, :512], OT[hh][:, qb * P:(qb + 1) * P],
                            wo_sb[:, hh, :],
                            start=(hh == 0), stop=(hh == HPC - 1))
                    osb = opool.tile([P, D], F32, name="osb", tag="osb")
                    nc.vector.tensor_copy(osb[:], pso[:, :512])
                    nc.sync.dma_start(out_p[qb * P:(qb + 1) * P, :], osb[:])

            def head_units(hl):            # noqa: C901
                pair, hp = hl // 2, hl % 2
                pslice = slice(hp * DK, (hp + 1) * DK)
                rec = small.tile([P, NQT], F32, name="rec", tag="rec")
                ot = otpool.tile([DK, S], F32R, name="ot", tag="OT")
                OT[hl] = ot

                def emit_l1(qb, hl=hl, pair=pair, pslice=pslice, rec=rec):
                    kl = klist[qb]
                    a_blk = apool.tile([P, S], F32, tag="A")
                    sums = small.tile([P, 4], F32, tag="sums")
                    nchunk = 0
                    pack = 0
                    for run in _runs(kl):
                        for c0 in range(0, len(run), CHUNK_BLKS):
                            chunk = run[c0:c0 + CHUNK_BLKS]
                            w = P * len(chunk)
                            pss = ps.tile([P, 1024], F32, tag="ps")
                            for m0 in range(0, w, 512):
                                mw = min(512, w - m0)
                                nc.tensor.matmul(
                                    pss[:, m0:m0 + mw],
                                    QT[pslice, pair, qb * P:(qb + 1) * P],
                                    KT[pslice, pair,
                                       chunk[0] * P + m0:
                                       chunk[0] * P + m0 + mw],
                                    start=True, stop=True)
                            for j, kb in enumerate(chunk):
                                if mixed[qb, kb]:
                                    mi = mix_idx[(qb, kb)]
                                    nc.vector.tensor_add(
                                        pss[:, j * P:(j + 1) * P],
                                        pss[:, j * P:(j + 1) * P],
                                        mm_sb[:, mi, :])
                            nc.scalar.activation(
                                a_blk[:, pack:pack + w], pss[:, :w], AF.Exp,
                                scale=SCALE,
                                accum_out=sums[:, nchunk:nchunk + 1])
                            nchunk += 1
                            pack += w
                    if nchunk == 1:
                        nc.vector.reciprocal(rec[:, qb:qb + 1], sums[:, 0:1])
                    else:
                        lsum = small.tile([P, 1], F32, tag="lsum")
                        nc.vector.reduce_sum(lsum[:], sums[:, :nchunk],
                                             axis=mybir.AxisListType.X)
                        nc.vector.reciprocal(rec[:, qb:qb + 1], lsum[:])
                    off = 0
                    for run in _runs(kl):
                        w = P * len(run)
                        nc.gpsimd.tensor_scalar_mul(
                            a_blk[:, off:off + w], a_blk[:, off:off + w],
                            rec[:, qb:qb + 1])
                        nc.sync.dma_start(
                            attn_o[hl, qb * P:(qb + 1) * P,
                                   run[0] * P: run[0] * P + w],
                            a_blk[:, off:off + w])
                        off += w

                def emit_r(half, rec=rec):
                    # per-q reciprocal rows for one q-half broadcast to
                    # [DK, HQ*P] via a DRAM bounce
                    qlo = half * HQ
                    r_sb = rpool.tile([DK, HQ * P], F32, tag="R")
                    prt = ps.tile([P, 1024], F32, tag="ps")
                    nc.tensor.transpose(prt[:HQ, :P], rec[:, qlo:qlo + HQ],
                                        ident[:])
                    rt_sb = small.tile([HQ, P], F32, tag="recT")
                    nc.vector.tensor_copy(rt_sb[:], prt[:HQ, :P])
                    rt_d = dram.tile([HQ, P], F32, tag="recTd")
                    nc.sync.dma_start(rt_d[:], rt_sb[:])
                    flat = rt_d[:].rearrange("a b -> (a b)")
                    bcast = bass.AP(tensor=flat.tensor, offset=flat.offset,
                                    ap=[[0, DK], list(flat.ap[0])])
                    nc.gpsimd.dma_start(r_sb[:], bcast)
                    return r_sb

                state = {}

                def emit_l2(kb, half, hl=hl, pair=pair, pslice=pslice,
                            state=state):
                    qlo, qhi = half * HQ, (half + 1) * HQ
                    ql = [qb for qb in qlist[kb] if qlo <= qb < qhi]
                    if ("po", half) not in state:
                        state[("po", half)] = po.tile(
                            [DK, HQ * P], F32, name="psum_o", tag="po")
                        state[("touched", half)] = set()
                    psum_o = state[("po", half)]
                    touched = state[("touched", half)]
                    at = atpool.tile([P, HQ * P], F32R, tag="AT")
                    pack = 0
                    packof = {}
                    for run in _runs(ql):
                        for c0 in range(0, len(run), CHUNK_BLKS):
                            chunk = run[c0:c0 + CHUNK_BLKS]
                            w = P * len(chunk)
                            pst = ps.tile([P, 1024], F32, tag="ps")
                            for m0 in range(0, w, 512):
                                mw = min(512, w - m0)
                                nc.tensor.matmul(
                                    pst[:, m0:m0 + mw],
                                    KT[pslice, pair, kb * P:(kb + 1) * P],
                                    QT[pslice, pair,
                                       chunk[0] * P + m0:
                                       chunk[0] * P + m0 + mw],
                                    start=True, stop=True)
                            for j, qb in enumerate(chunk):
                                if mixed[qb, kb]:
                                    mi = mix_idx[(qb, kb)]
                                    nc.vector.tensor_add(
                                        pst[:, j * P:(j + 1) * P],
                                        pst[:, j * P:(j + 1) * P],
                                        mmT_sb[:, mi, :])
                            nc.scalar.activation(
                                at[:, pack:pack + w], pst[:, :w], AF.Exp,
                                scale=SCALE)
                            for j, qb in enumerate(chunk):
                                packof[qb] = pack + j * P
                            pack += w
                    # PV: segment runs by uniform (start, stop) flags and
                    # by PSUM bank (4 blocks = 512 f32 cols per bank)
                    for run in _runs(ql):
                        seg = []
                        for qb in run + [None]:
                            key = (None if qb is None
                                   else (first_kb[qb] == kb,
                                         last_kb[qb] == kb, qb // 4))
                            if seg and (qb is None or key != seg[0][1]):
                                qs = [q for q, _ in seg]
                                st_, sp_, _ = seg[0][1]
                                wseg = P * len(qs)
                                q0 = (qs[0] - qlo) * P
                                nc.tensor.matmul(
                                    psum_o[:, q0: q0 + wseg],
                                    V[:, kb, hl * DK:(hl + 1) * DK],
                                    at[:, packof[qs[0]]:
                                       packof[qs[0]] + wseg],
                                    start=st_, stop=sp_,
                                    skip_group_check=True)
                                touched.update(qs)
                                seg = []
                            if qb is not None:
                                seg.append((qb, key))

                def emit_fin(half, r_sb, ot=ot, state=state):
                    # normalize O^T rows, store f32r for the out projection
                    qlo, qhi = half * HQ, (half + 1) * HQ
                    psum_o = state[("po", half)]
                    touched = state[("touched", half)]
                    nc.vector.tensor_mul(ot[:, qlo * P: qhi * P], psum_o[:],
                                         r_sb[:])
                    for qb in range(qlo, qhi):
                        if qb not in touched:
                            nc.vector.memset(ot[:, qb * P:(qb + 1) * P], 0.0)

                # per q-half: weave loop1 and loop2 units, then R + finalize
                def sg_l1(qb):
                    return ((qb + 1) * P + SG - 1) // SG - 1

                out = []
                for half in range(2):
                    qlo, qhi = half * HQ, (half + 1) * HQ
                    l1u = [(sg_l1(qb), lambda qb=qb: emit_l1(qb))
                           for qb in range(qlo, qhi) if klist[qb]]
                    sg_h = (qhi * P + SG - 1) // SG - 1
                    l2u = [
                        (sg_h, lambda kb=kb, half=half: emit_l2(kb, half))
                        for kb in range(NQT)
                        if any(qlo <= qb < qhi for qb in qlist[kb])
                    ]
                    woven = []
                    n1, n2 = len(l1u), len(l2u)
                    i1 = i2 = 0
                    while i1 < n1 or i2 < n2:
                        if i1 < n1 and (i2 >= n2 or i1 * n2 <= i2 * n1):
                            woven.append(l1u[i1])
                            i1 += 1
                        else:
                            woven.append(l2u[i2])
                            i2 += 1
                    woven.append(
                        (sg_h,
                         lambda half=half: emit_fin(half, emit_r(half))))
                    out += woven
                return out

            for hl in range(HPC):
                units += head_units(hl)

            ui = 0
            for sg in range(NSG):
                emit_proj(sg)
                emit_proj_v(sg)
                while ui < len(units) and units[ui][0] <= sg:
                    units[ui][1]()
                    ui += 1
            while ui < len(units):
                units[ui][1]()
                ui += 1
            emit_outproj(0)
            emit_outproj(1)


    nc.compile()
    return nc


# ---------------------------------------------------------------- host side

def _prep_core_inputs(inputs, core, n_mix, mix_np, mixT_np):
    b, g = core // 2, core % 2
    hsel = slice(g * HPC * DK, (g + 1) * HPC * DK)
    q = np.asarray(inputs["query"], np.float32)[b]
    k = np.asarray(inputs["key"], np.float32)[b]
    v = np.asarray(inputs["value"], np.float32)[b]

    def t_in(x):   # [S, D] -> [NDC, P, S] fp32 (transposed, chunked)
        return np.ascontiguousarray(x.T).reshape(NDC, P, S)

    def w_in(w):   # [D, 256] -> [NDC, P, 256]
        return np.ascontiguousarray(w[:, hsel]).reshape(NDC, P, HPC * DK)

    wo = np.asarray(inputs["Wo"], np.float32)[hsel, :]          # [256, D]
    bq = np.asarray(inputs["bq"], np.float32)[hsel].reshape(2, P).T
    bk = np.asarray(inputs["bk"], np.float32)[hsel].reshape(2, P).T
    m = {
        "qT": t_in(q), "kT": t_in(k), "vT": t_in(v),
        "wq": w_in(np.asarray(inputs["Wq"], np.float32)),
        "wk": w_in(np.asarray(inputs["Wk"], np.float32)),
        "wv": w_in(np.asarray(inputs["Wv"], np.float32)),
        "wo": np.ascontiguousarray(wo).reshape(HPC, DK, D),
        "bq2": np.ascontiguousarray(bq),
        "bk2": np.ascontiguousarray(bk),
    }
    if n_mix:
        m["mmix"] = mix_np
        m["mmixT"] = mixT_np
    return m


def _get_program(mask2d):
    key = mask2d.tobytes()
    if _CACHE.get("key") != key:
        nz, mixed, mix_idx, mix_np, mixT_np = _mask_structure(mask2d)
        nc = _build_program(nz, mixed, mix_idx, len(mix_np))
        _CACHE.update(key=key, nc=nc, mix_np=mix_np, mixT_np=mixT_np,
                      n_mix=len(mix_np))
    return _CACHE


def run(inputs, trace=False):
    mask2d = np.asarray(inputs["mask"], np.int32).reshape(S, S)
    prog = _get_program(mask2d)
    in_maps = [
        _prep_core_inputs(inputs, c, prog["n_mix"], prog["mix_np"],
                          prog["mixT_np"])
        for c in range(NCORES)
    ]
    res = bass_utils.run_bass_kernel_spmd(
        prog["nc"], in_maps, core_ids=list(range(NCORES)), trace=trace)

    attn = np.empty((B, H, S, S), np.float32)
    out = np.empty((B, S, D), np.float32)
    bvWo_bo = (
        np.asarray(inputs["bv"], np.float32) @ np.asarray(inputs["Wo"], np.float32)
        + np.asarray(inputs["bo"], np.float32)
    )
    for b in range(B):
        for g in range(2):
            attn[b, g * HPC:(g + 1) * HPC] = res.results[2 * b + g]["attn_o"]
        out[b] = (res.results[2 * b]["out_p"] + res.results[2 * b + 1]["out_p"]
                  + bvWo_bo)
    return (out, attn), res


def kernel(**inputs):
    (out, attn), _ = run(inputs)
    return (out, attn)


# revision 30
# speedup vs baseline: 1.0019x; 1.0019x over previous
"""Trainium2 Bass kernel for causal multi-head attention (B=4, S=2048, D=512, H=8).

Returns (out, attn) like the reference. Sharding: 8 cores = 4 batches x 2
head-groups; each core handles 1 batch x 4 heads. The [B,H,Sq,Sk] attn tensor
shards naturally along (B, H); the output projection is computed per-core as a
partial over its 4 heads and summed on the host during the gather.

All matmul operands use float32r (fp32 storage, reduced-precision multiply,
full-rate on the PE at free-dim >= 256); softmax runs in fp32 on the scalar
engine (exp with accumulated row sums). The causal mask is exploited
structurally: all-zero 128x128 blocks of the mask are skipped entirely (the
output buffer is zero-initialized), all-ones blocks skip masking, and mixed
blocks get an additive -1e30 mask before exp.

Self-contained: hardcodes shapes; builds + compiles the Bass program on first
call, runs via bass_utils.run_bass_kernel_spmd on cores 0-7.
"""

import numpy as np

import concourse.bass as bass
import concourse.mybir as mybir
import concourse.tile as tile
from concourse import bacc, bass_utils
from concourse.masks import make_identity

B, S, D, H = 4, 2048, 512, 8
DK = D // H                    # 64 head dim
NCORES = 8
HPC = H // 2                   # 4 heads per core
P = 128                        # partition tile
NQT = S // P                   # 16 row/col tiles
NDC = D // P                   # 4 input-dim chunks
CHUNK_BLKS = 8                 # 8x128 = 1024-wide PSUM chunks (2 banks)
SG = 256                       # projection s-group width
SCALE = 1.0 / 8.0              # 1/sqrt(DK)
NEG = -1.0e30

F32 = mybir.dt.float32
F32R = mybir.dt.float32r
BF = mybir.dt.bfloat16
AF = mybir.ActivationFunctionType

_CACHE: dict = {}


# ---------------------------------------------------------------- mask prep

def _mask_structure(mask2d: np.ndarray):
    """Classify each 128x128 block of the [S,S] mask."""
    blk = mask2d.reshape(NQT, P, NQT, P).transpose(0, 2, 1, 3)
    nz = blk.any(axis=(2, 3))
    allone = blk.all(axis=(2, 3))
    mixed = nz & ~allone
    mix_idx = {}
    mix_add, mixT_add = [], []
    for qb in range(NQT):
        for kb in range(NQT):
            if mixed[qb, kb]:
                mix_idx[(qb, kb)] = len(mix_add)
                add = np.where(blk[qb, kb] == 0, np.float32(NEG), np.float32(0.0))
                mix_add.append(add)
                mixT_add.append(np.ascontiguousarray(add.T))
    n = len(mix_add)
    mix_np = np.stack(mix_add) if n else np.zeros((0, P, P), np.float32)
    mixT_np = np.stack(mixT_add) if n else np.zeros((0, P, P), np.float32)
    return nz, mixed, mix_idx, mix_np, mixT_np


def _runs(idxs):
    """Split a sorted index list into maximal consecutive runs."""
    runs = []
    for i in idxs:
        if runs and i == runs[-1][-1] + 1:
            runs[-1].append(i)
        else:
            runs.append([i])
    return runs


# ---------------------------------------------------------------- program

def _build_program(nz, mixed, mix_idx, n_mix):
    nc = bacc.Bacc("TRN2", target_bir_lowering=False, debug=False,
                   num_devices=NCORES)

    # per-core external IO (f32r tensors carry plain fp32 bytes)
    qT_d = nc.dram_tensor("qT", [NDC, P, S], F32R, kind="ExternalInput")
    kT_d = nc.dram_tensor("kT", [NDC, P, S], F32R, kind="ExternalInput")
    vT_d = nc.dram_tensor("vT", [NDC, P, S], F32R, kind="ExternalInput")
    wq_d = nc.dram_tensor("wq", [NDC, P, HPC * DK], F32R, kind="ExternalInput")
    wk_d = nc.dram_tensor("wk", [NDC, P, HPC * DK], F32R, kind="ExternalInput")
    wv_d = nc.dram_tensor("wv", [NDC, P, HPC * DK], F32R, kind="ExternalInput")
    wo_d = nc.dram_tensor("wo", [HPC, DK, D], F32R, kind="ExternalInput")
    bq_d = nc.dram_tensor("bq2", [P, 2], F32, kind="ExternalInput")
    bk_d = nc.dram_tensor("bk2", [P, 2], F32, kind="ExternalInput")
    if n_mix:
        mm_d = nc.dram_tensor("mmix", [n_mix, P, P], F32, kind="ExternalInput")
        mmT_d = nc.dram_tensor("mmixT", [n_mix, P, P], F32, kind="ExternalInput")
    attn_o = nc.dram_tensor("attn_o", [HPC, S, S], F32, kind="ExternalOutput")
    out_p = nc.dram_tensor("out_p", [S, D], F32, kind="ExternalOutput")

    klist = [[kb for kb in range(NQT) if nz[qb, kb]] for qb in range(NQT)]
    qlist = [[qb for qb in range(NQT) if nz[qb, kb]] for kb in range(NQT)]
    first_kb = {qb: klist[qb][0] for qb in range(NQT) if klist[qb]}
    last_kb = {qb: klist[qb][-1] for qb in range(NQT) if klist[qb]}

    with tile.TileContext(nc) as tc:
        with (
            tc.tile_pool(name="const", bufs=1) as const,
            tc.tile_pool(name="stream", bufs=2) as stream,
            tc.tile_pool(name="proj", bufs=1) as proj,
            tc.tile_pool(name="apool", bufs=3) as apool,
            tc.tile_pool(name="atpool", bufs=4) as atpool,
            tc.tile_pool(name="otpool", bufs=4) as otpool,
            tc.tile_pool(name="rpool", bufs=2) as rpool,
            tc.tile_pool(name="small", bufs=6) as small,
            tc.tile_pool(name="opool", bufs=3) as opool,
            tc.tile_pool(name="ps", bufs=3, space="PSUM") as ps,
            tc.tile_pool(name="po", bufs=1, space="PSUM") as po,
            tc.tile_pool(name="dram", bufs=4, space="DRAM") as dram,
        ):
            # ---- constants
            wq_sb = const.tile([P, NDC, HPC * DK], F32R, tag="wq")
            wk_sb = const.tile([P, NDC, HPC * DK], F32R, tag="wk")
            wv_sb = const.tile([P, NDC, HPC * DK], F32R, tag="wv")
            wo_sb = const.tile([DK, HPC, D], F32R, tag="wo")
            bq_sb = const.tile([P, 2], F32, tag="bq")
            bk_sb = const.tile([P, 2], F32, tag="bk")
            ident = const.tile([P, P], F32, tag="ident")
            nc.sync.dma_start(wq_sb[:], wq_d[:].rearrange("c p m -> p c m"))
            nc.sync.dma_start(wk_sb[:], wk_d[:].rearrange("c p m -> p c m"))
            nc.sync.dma_start(wv_sb[:], wv_d[:].rearrange("c p m -> p c m"))
            nc.sync.dma_start(wo_sb[:], wo_d[:].rearrange("h p d -> p h d"))
            nc.sync.dma_start(bq_sb[:], bq_d[:])
            nc.sync.dma_start(bk_sb[:], bk_d[:])
            make_identity(nc, ident[:])
            ident_r = const.tile([P, P], F32R, tag="ident_r")
            nc.vector.tensor_copy(ident_r[:], ident[:])
            if n_mix:
                mm_sb = const.tile([P, n_mix, P], F32R, tag="mm")
                mmT_sb = const.tile([P, n_mix, P], F32R, tag="mmT")
                nc.gpsimd.dma_start(mm_sb[:], mm_d[:].rearrange("n p k -> p n k"))
                nc.gpsimd.dma_start(mmT_sb[:], mmT_d[:].rearrange("n p k -> p n k"))

            # ---- projections, streaming the transposed inputs per s-group
            QT = proj.tile([P, 2, S], F32R, tag="QT")
            KT = proj.tile([P, 2, S], F32R, tag="KT")
            V = proj.tile([P, NQT, HPC * DK], F32R, tag="V")
            NSG = S // SG

            def emit_proj_v(sg):
                sl = slice(sg * SG, (sg + 1) * SG)
                vts = stream.tile([P, NDC, SG], F32R, tag="vts")
                nc.sync.dma_start(vts[:], vT_d[:, :, sl].rearrange("c p s -> p c s"))
                for t in range(SG // P):
                    st = sg * (SG // P) + t
                    pv = ps.tile([P, 1024], F32, tag="ps")
                    for c in range(NDC):
                        nc.tensor.matmul(
                            pv[:, :HPC * DK], vts[:, c, t * P:(t + 1) * P],
                            wv_sb[:, c, :], start=(c == 0), stop=(c == NDC - 1))
                    nc.vector.tensor_copy(V[:, st, :], pv[:, :HPC * DK])

            def emit_proj(sg):
                sl = slice(sg * SG, (sg + 1) * SG)
                qts = stream.tile([P, NDC, SG], F32R, tag="qts")
                kts = stream.tile([P, NDC, SG], F32R, tag="kts")
                nc.sync.dma_start(qts[:], qT_d[:, :, sl].rearrange("c p s -> p c s"))
                nc.sync.dma_start(kts[:], kT_d[:, :, sl].rearrange("c p s -> p c s"))
                for pair in range(2):
                    psl = slice(pair * P, (pair + 1) * P)
                    pq = ps.tile([P, 1024], F32, tag="ps")
                    pk = ps.tile([P, 1024], F32, tag="ps")
                    for c in range(NDC):
                        nc.tensor.matmul(pq[:, :SG], wq_sb[:, c, psl], qts[:, c, :],
                                         start=(c == 0), stop=(c == NDC - 1))
                    nc.vector.tensor_scalar_add(QT[:, pair, sl], pq[:, :SG],
                                                bq_sb[:, pair:pair + 1])
                    for c in range(NDC):
                        nc.tensor.matmul(pk[:, :SG], wk_sb[:, c, psl], kts[:, c, :],
                                         start=(c == 0), stop=(c == NDC - 1))
                    nc.vector.tensor_scalar_add(KT[:, pair, sl], pk[:, :SG],
                                                bk_sb[:, pair:pair + 1])

            # ---- attention emission units. Each unit carries the highest
            # projection s-group it reads so head 0 can interleave with the
            # projection/input-load stream; within each (head, q-half) the
            # loop1 (attn output) and loop2 (A^T + PV) streams are woven so
            # every engine's static order alternates between them.
            OT = [None] * HPC
            HQ = NQT // 2
            units = []                     # (need_sg, emit_fn) in order

            def emit_outproj(half):
                # out[q,:] = sum_h OT_h[:,q].T @ wo_h for this q-half
                for qb in range(half * HQ, (half + 1) * HQ):
                    pso = ps.tile([P, 1024], F32, name="pso", tag="ps")
                    for hh in range(HPC):
                        nc.tensor.matmul(
                            pso[:, :512], OT[hh][:, qb * P:(qb + 1) * P],
                            wo_sb[:, hh, :],
                            start=(hh == 0), stop=(hh == HPC - 1))
                    osb = opool.tile([P, D], F32, name="osb", tag="osb")
                    nc.vector.tensor_copy(osb[:], pso[:, :512])
                    nc.sync.dma_start(out_p[qb * P:(qb + 1) * P, :], osb[:])

            def head_units(hl):            # noqa: C901
                pair, hp = hl // 2, hl % 2
                pslice = slice(hp * DK, (hp + 1) * DK)
                rec = small.tile([P, NQT], F32, name="rec", tag="rec")
                ot = otpool.tile([DK, S], F32R, name="ot", tag="OT")
                OT[hl] = ot

                def emit_l1(qb, hl=hl, pair=pair, pslice=pslice, rec=rec):
                    kl = klist[qb]
                    a_blk = apool.tile([P, S], F32, tag="A")
                    sums = small.tile([P, 4], F32, tag="sums")
                    nchunk = 0
                    pack = 0
                    for run in _runs(kl):
                        for c0 in range(0, len(run), CHUNK_BLKS):
                            chunk = run[c0:c0 + CHUNK_BLKS]
                            w = P * len(chunk)
                            pss = ps.tile([P, 1024], F32, tag="ps")
                            for m0 in range(0, w, 512):
                                mw = min(512, w - m0)
                                nc.tensor.matmul(
                                    pss[:, m0:m0 + mw],
                                    QT[pslice, pair, qb * P:(qb + 1) * P],
                                    KT[pslice, pair,
                                       chunk[0] * P + m0:
                                       chunk[0] * P + m0 + mw],
                                    start=True, stop=False,
                                    skip_group_check=True)
                            for j, kb in enumerate(chunk):
                                if mixed[qb, kb]:
                                    mi = mix_idx[(qb, kb)]
                                    nc.tensor.matmul(
                                        pss[:, j * P:(j + 1) * P],
                                        ident_r[:], mm_sb[:, mi, :],
                                        start=False, stop=True,
                                        skip_group_check=True)
                            nc.scalar.activation(
                                a_blk[:, pack:pack + w], pss[:, :w], AF.Exp,
                                scale=SCALE,
                                accum_out=sums[:, nchunk:nchunk + 1])
                            nchunk += 1
                            pack += w
                    if nchunk == 1:
                        nc.vector.reciprocal(rec[:, qb:qb + 1], sums[:, 0:1])
                    else:
                        lsum = small.tile([P, 1], F32, tag="lsum")
                        nc.vector.reduce_sum(lsum[:], sums[:, :nchunk],
                                             axis=mybir.AxisListType.X)
                        nc.vector.reciprocal(rec[:, qb:qb + 1], lsum[:])
                    off = 0
                    for run in _runs(kl):
                        w = P * len(run)
                        nc.gpsimd.tensor_scalar_mul(
                            a_blk[:, off:off + w], a_blk[:, off:off + w],
                            rec[:, qb:qb + 1])
                        nc.sync.dma_start(
                            attn_o[hl, qb * P:(qb + 1) * P,
                                   run[0] * P: run[0] * P + w],
                            a_blk[:, off:off + w])
                        off += w

                def emit_r(half, rec=rec):
                    # per-q reciprocal rows for one q-half broadcast to
                    # [DK, HQ*P] via a DRAM bounce
                    qlo = half * HQ
                    r_sb = rpool.tile([DK, HQ * P], F32, tag="R")
                    prt = ps.tile([P, 1024], F32, tag="ps")
                    nc.tensor.transpose(prt[:HQ, :P], rec[:, qlo:qlo + HQ],
                                        ident[:])
                    rt_sb = small.tile([HQ, P], F32, tag="recT")
                    nc.vector.tensor_copy(rt_sb[:], prt[:HQ, :P])
                    rt_d = dram.tile([HQ, P], F32, tag="recTd")
                    nc.sync.dma_start(rt_d[:], rt_sb[:])
                    flat = rt_d[:].rearrange("a b -> (a b)")
                    bcast = bass.AP(tensor=flat.tensor, offset=flat.offset,
                                    ap=[[0, DK], list(flat.ap[0])])
                    nc.gpsimd.dma_start(r_sb[:], bcast)
                    return r_sb

                state = {}

                def emit_l2(kb, half, hl=hl, pair=pair, pslice=pslice,
                            state=state):
                    qlo, qhi = half * HQ, (half + 1) * HQ
                    ql = [qb for qb in qlist[kb] if qlo <= qb < qhi]
                    if ("po", half) not in state:
                        state[("po", half)] = po.tile(
                            [DK, HQ * P], F32, name="psum_o", tag="po")
                        state[("touched", half)] = set()
                    psum_o = state[("po", half)]
                    touched = state[("touched", half)]
                    at = atpool.tile([P, HQ * P], F32R, tag="AT")
                    pack = 0
                    packof = {}
                    for run in _runs(ql):
                        for c0 in range(0, len(run), CHUNK_BLKS):
                            chunk = run[c0:c0 + CHUNK_BLKS]
                            w = P * len(chunk)
                            pst = ps.tile([P, 1024], F32, tag="ps")
                            for m0 in range(0, w, 512):
                                mw = min(512, w - m0)
                                nc.tensor.matmul(
                                    pst[:, m0:m0 + mw],
                                    KT[pslice, pair, kb * P:(kb + 1) * P],
                                    QT[pslice, pair,
                                       chunk[0] * P + m0:
                                       chunk[0] * P + m0 + mw],
                                    start=True, stop=False,
                                    skip_group_check=True)
                            for j, qb in enumerate(chunk):
                                if mixed[qb, kb]:
                                    mi = mix_idx[(qb, kb)]
                                    nc.tensor.matmul(
                                        pst[:, j * P:(j + 1) * P],
                                        ident_r[:], mmT_sb[:, mi, :],
                                        start=False, stop=True,
                                        skip_group_check=True)
                            nc.scalar.activation(
                                at[:, pack:pack + w], pst[:, :w], AF.Exp,
                                scale=SCALE)
                            for j, qb in enumerate(chunk):
                                packof[qb] = pack + j * P
                            pack += w
                    # PV: segment runs by uniform (start, stop) flags and
                    # by PSUM bank (4 blocks = 512 f32 cols per bank)
                    for run in _runs(ql):
                        seg = []
                        for qb in run + [None]:
                            key = (None if qb is None
                                   else (first_kb[qb] == kb,
                                         last_kb[qb] == kb, qb // 4))
                            if seg and (qb is None or key != seg[0][1]):
                                qs = [q for q, _ in seg]
                                st_, sp_, _ = seg[0][1]
                                wseg = P * len(qs)
                                q0 = (qs[0] - qlo) * P
                                nc.tensor.matmul(
                                    psum_o[:, q0: q0 + wseg],
                                    V[:, kb, hl * DK:(hl + 1) * DK],
                                    at[:, packof[qs[0]]:
                                       packof[qs[0]] + wseg],
                                    start=st_, stop=sp_,
                                    skip_group_check=True)
                                touched.update(qs)
                                seg = []
                            if qb is not None:
                                seg.append((qb, key))

                def emit_fin(half, r_sb, ot=ot, state=state):
                    # normalize O^T rows, store f32r for the out projection
                    qlo, qhi = half * HQ, (half + 1) * HQ
                    psum_o = state[("po", half)]
                    touched = state[("touched", half)]
                    nc.vector.tensor_mul(ot[:, qlo * P: qhi * P], psum_o[:],
                                         r_sb[:])
                    for qb in range(qlo, qhi):
                        if qb not in touched:
                            nc.vector.memset(ot[:, qb * P:(qb + 1) * P], 0.0)

                # per q-half: weave loop1 and loop2 units, then R + finalize
                def sg_l1(qb):
                    return ((qb + 1) * P + SG - 1) // SG - 1

                out = []
                for half in range(2):
                    qlo, qhi = half * HQ, (half + 1) * HQ
                    l1u = [(sg_l1(qb), lambda qb=qb: emit_l1(qb))
                           for qb in range(qlo, qhi) if klist[qb]]
                    sg_h = (qhi * P + SG - 1) // SG - 1
                    l2u = [
                        (sg_h, lambda kb=kb, half=half: emit_l2(kb, half))
                        for kb in range(NQT)
                        if any(qlo <= qb < qhi for qb in qlist[kb])
                    ]
                    woven = []
                    n1, n2 = len(l1u), len(l2u)
                    i1 = i2 = 0
                    while i1 < n1 or i2 < n2:
                        if i1 < n1 and (i2 >= n2 or i1 * n2 <= i2 * n1):
                            woven.append(l1u[i1])
                            i1 += 1
                        else:
                            woven.append(l2u[i2])
                            i2 += 1
                    woven.append(
                        (sg_h,
                         lambda half=half: emit_fin(half, emit_r(half))))
                    out += woven
                return out

            for hl in range(HPC):
                units += head_units(hl)

            ui = 0
            for sg in range(NSG):
                emit_proj(sg)
                emit_proj_v(sg)
                while ui < len(units) and units[ui][0] <= sg:
                    units[ui][1]()
                    ui += 1
            while ui < len(units):
                units[ui][1]()
                ui += 1
            emit_outproj(0)
            emit_outproj(1)


    nc.compile()
    return nc


# ---------------------------------------------------------------- host side

def _prep_core_inputs(inputs, core, n_mix, mix_np, mixT_np):
    b, g = core // 2, core % 2
    hsel = slice(g * HPC * DK, (g + 1) * HPC * DK)
    q = np.asarray(inputs["query"], np.float32)[b]
    k = np.asarray(inputs["key"], np.float32)[b]
    v = np.asarray(inputs["value"], np.float32)[b]

    def t_in(x):   # [S, D] -> [NDC, P, S] fp32 (transposed, chunked)
        return np.ascontiguousarray(x.T).reshape(NDC, P, S)

    def w_in(w):   # [D, 256] -> [NDC, P, 256]
        return np.ascontiguousarray(w[:, hsel]).reshape(NDC, P, HPC * DK)

    wo = np.asarray(inputs["Wo"], np.float32)[hsel, :]          # [256, D]
    bq = np.asarray(inputs["bq"], np.float32)[hsel].reshape(2, P).T
    bk = np.asarray(inputs["bk"], np.float32)[hsel].reshape(2, P).T
    m = {
        "qT": t_in(q), "kT": t_in(k), "vT": t_in(v),
        "wq": w_in(np.asarray(inputs["Wq"], np.float32)),
        "wk": w_in(np.asarray(inputs["Wk"], np.float32)),
        "wv": w_in(np.asarray(inputs["Wv"], np.float32)),
        "wo": np.ascontiguousarray(wo).reshape(HPC, DK, D),
        "bq2": np.ascontiguousarray(bq),
        "bk2": np.ascontiguousarray(bk),
    }
    if n_mix:
        m["mmix"] = mix_np
        m["mmixT"] = mixT_np
    return m


def _get_program(mask2d):
    key = mask2d.tobytes()
    if _CACHE.get("key") != key:
        nz, mixed, mix_idx, mix_np, mixT_np = _mask_structure(mask2d)
        nc = _build_program(nz, mixed, mix_idx, len(mix_np))
        _CACHE.update(key=key, nc=nc, mix_np=mix_np, mixT_np=mixT_np,
                      n_mix=len(mix_np))
    return _CACHE


def run(inputs, trace=False):
    mask2d = np.asarray(inputs["mask"], np.int32).reshape(S, S)
    prog = _get_program(mask2d)
    in_maps = [
        _prep_core_inputs(inputs, c, prog["n_mix"], prog["mix_np"],
                          prog["mixT_np"])
        for c in range(NCORES)
    ]
    res = bass_utils.run_bass_kernel_spmd(
        prog["nc"], in_maps, core_ids=list(range(NCORES)), trace=trace)

    attn = np.empty((B, H, S, S), np.float32)
    out = np.empty((B, S, D), np.float32)
    bvWo_bo = (
        np.asarray(inputs["bv"], np.float32) @ np.asarray(inputs["Wo"], np.float32)
        + np.asarray(inputs["bo"], np.float32)
    )
    for b in range(B):
        for g in range(2):
            attn[b, g * HPC:(g + 1) * HPC] = res.results[2 * b + g]["attn_o"]
        out[b] = (res.results[2 * b]["out_p"] + res.results[2 * b + 1]["out_p"]
                  + bvWo_bo)
    return (out, attn), res


def kernel(**inputs):
    (out, attn), _ = run(inputs)
    return (out, attn)
